# revision 21
# baseline (speedup 1.0000x reference)
"""Trainium2 Bass kernel for a GNN message-passing encoder (PocketGraphEncoder).

Math (matches the reference):
    x   = relu(node_scalar @ W_in + b_in)                  # (N, H)
    agg = segment_mean(x[src] over dst, clamp deg>=1)      # (N, H)
    h   = relu(x @ W_self + b_self + agg @ W_nei + b_nei)  # (N, H)
    out = h.mean(0) @ W_out + b_out                        # (OUT,)

Strategy (8 cores, no collectives):
  * Nodes are dealt round-robin from a global in-degree sort, so every core
    owns N/8 nodes with a near-identical degree profile and all cores run one
    shared program (SPMD differs only in input data).
  * Phase 1: every core computes full x (bf16, N rows) into its DRAM, stored
    in "pages" of <=32768 rows so row indices fit int16 for the fast
    dma_gather ucode.  The bias is folded into the matmul via a ones-row.
  * Phase 4 (per page g): core nodes are sorted by their page-g in-degree;
    pass k gathers the k-th page-g in-neighbor's x row for the first M_gk
    nodes.  Degree-sorted order makes destinations a prefix -> scatter-free:
    dma_gather lands rows in accumulator order, DVE adds accumulate.
    Per-page partial sums (in page-g node order) are spilled to DRAM; the
    last page's order IS the main order.  A second tiny dma_gather permutes
    each spilled partial back into main order for the final combine.
  * Phase 5: scale by 1/deg, PE-transpose agg tiles, compute
    h^T = relu(W_self^T x^T + W_nei^T agg^T + b) and reduce along nodes with
    the ACT accumulator -> per-core partial (256,) sum.
  * Host: sum partials, /N, final tiny GEMM with W_out.
"""

import os

import ml_dtypes
import numpy as np

import concourse.bacc as bacc
import concourse.mybir as mybir
import concourse.tile as tile

P = 128
N_CORES = 8
CHUNK = 512  # nodes per phase-5 chunk
CHUNK1 = 1024  # nodes per phase-1 chunk
GCOLS = 16  # gather columns (x128 rows) per dma_gather piece
GCOLS_C = 8  # combine gather piece (f32 staging)
PCAP = 31744  # real x rows per page (multiple of CHUNK1)

F32 = mybir.dt.float32
BF16 = mybir.dt.bfloat16
I16 = mybir.dt.int16
NP_BF16 = ml_dtypes.bfloat16
X_BF16 = True  # store x pages (and phase-1 math) in bf16


def _xdt():
    return BF16 if X_BF16 else F32


def _np_xdt():
    return NP_BF16 if X_BF16 else np.float32

RELU = mybir.ActivationFunctionType.Relu


def _ceil_div(a, b):
    return (a + b - 1) // b


def _wrap16(vals):
    """int16 index list -> [128, n/16] dma_gather table layout."""
    n = len(vals)
    assert n % 16 == 0
    tab = np.asarray(vals, np.int16).reshape(n // 16, 16).T  # [16, n/16]
    return np.tile(tab, (8, 1))  # replicated for the 8 Q7 cores


def _prep(node_scalar, edge_index, n_cores):
    """Host-side preprocessing. Returns (cfg, shared_inputs, per_core_inputs)."""
    N, IN = node_scalar.shape
    src = np.asarray(edge_index[0], dtype=np.int64)
    dst = np.asarray(edge_index[1], dtype=np.int64)

    PAGE = PCAP + CHUNK1  # storage rows per page (<= 32768 for int16)
    assert PAGE <= 32768 and PCAP % CHUNK1 == 0

    deg = np.bincount(dst, minlength=N).astype(np.int64)

    # node -> core: round-robin deal from global total-degree sort
    rank_order = np.argsort(-deg, kind="stable")
    assert N % n_cores == 0
    NV = N // n_cores
    core_nodes = [rank_order[c::n_cores] for c in range(n_cores)]

    NODE_CH = _ceil_div(NV, CHUNK)
    NV_PAD = NODE_CH * CHUNK
    AGG_COLS = NV_PAD // P

    # phase-1 layout: compute x for v in [0, NPAD1); v = N.. are exact zeros
    # (ones-row zeroed there).  npages covers all computed rows.
    N1CH = _ceil_div(N + 1, CHUNK1)
    NPAD1 = N1CH * CHUNK1
    npages = _ceil_div(NPAD1, PCAP)
    X_ROWS = npages * PAGE

    page_of = src // PCAP
    src_local = (src - page_of * PCAP).astype(np.int32)

    # zero row (page-local) per page: prefer a computed-zero row; else a
    # dedicated zero chunk at local PCAP.
    zero_local = []
    zero_chunk_pages = []
    for g in range(npages):
        lo, hi = g * PCAP, (g + 1) * PCAP
        if N < hi and NPAD1 > max(N, lo):
            zero_local.append(max(N, lo) - lo)
        else:
            zero_local.append(PCAP)
            zero_chunk_pages.append(g)

    # phase-1 chunk list: (ns_t column start, x_full storage row start)
    NS_COLS = NPAD1 + len(zero_chunk_pages) * CHUNK1
    page_chunks = [[] for _ in range(npages)]
    for zi, g in enumerate(zero_chunk_pages):
        page_chunks[g].append((NPAD1 + zi * CHUNK1, PCAP))
    for ci in range(N1CH):
        v0 = ci * CHUNK1
        g = v0 // PCAP
        page_chunks[g].append((v0, v0 - g * PCAP))

    # per-(node, page) in-edge lists: group edges by (dst, page)
    key = dst * npages + page_of
    deg_np = np.bincount(key, minlength=N * npages).reshape(N, npages)
    order2 = np.argsort(key, kind="stable")
    src_by_dp = src_local[order2]
    start2 = np.zeros(N * npages + 1, dtype=np.int64)
    np.cumsum(deg_np.reshape(-1), out=start2[1:])

    # per-core, per-page node orders (by page-degree desc; stable)
    # main order = last page's order (its accumulation stays in SBUF).
    orders = []  # [core][page] -> node ids in page order
    for c in range(n_cores):
        cn = core_nodes[c]
        ords = []
        for g in range(npages):
            dg = deg_np[cn, g]
            ords.append(cn[np.argsort(-dg, kind="stable")])
        orders.append(ords)

    # uniform pass lengths per page: M_gk = max over cores of m_gk
    pass_cols = []  # [page][pass] -> cols (128-row units)
    for g in range(npages):
        maxd = 0
        for c in range(n_cores):
            maxd = max(maxd, int(deg_np[core_nodes[c], g].max(initial=0)))
        cols_k = []
        for k in range(maxd):
            m = 0
            for c in range(n_cores):
                m = max(m, int(np.count_nonzero(deg_np[core_nodes[c], g] > k)))
            if m == 0:
                break
            cols_k.append(_ceil_div(m, P))
        pass_cols.append(cols_k)

    # static piece lists (shared by all cores)
    # gather pieces: (page, table col0, ncols(128-row units), agg col0)
    gpieces = []
    page_tab_cols = []  # int16-table column extent per page
    tcol = 0
    for g in range(npages):
        t0 = tcol
        for cols in pass_cols[g]:
            j = 0
            while j < cols:
                nc_ = min(GCOLS, cols - j)
                gpieces.append((g, tcol - t0, nc_, j))
                tcol += nc_ * 8  # 128 idx per col = 8 int16-table cols
                j += nc_
        page_tab_cols.append(tcol - t0)
    # combine pieces: (page, table col0, ncols, agg col0) on the perm table
    cpieces = []
    comb_tab_cols = AGG_COLS * 8
    j = 0
    while j < AGG_COLS:
        nc_ = min(GCOLS_C, AGG_COLS - j)
        for g in range(npages - 1):
            cpieces.append((g, j * 8, nc_, j))
        j += nc_

    # per-core tables + inputs
    per_core = []
    for c in range(n_cores):
        gtab_parts = []
        for g in range(npages):
            zl = zero_local[g]
            cn_g = orders[c][g]
            dg = deg_np[cn_g, g]
            base = start2[cn_g * npages + g]
            for k, cols in enumerate(pass_cols[g]):
                m = int(np.count_nonzero(dg > k))
                block = np.full((cols * P,), zl, dtype=np.int32)
                if m > 0:
                    block[:m] = src_by_dp[base[:m] + k]
                gtab_parts.append(block)
        gtab16 = _wrap16(np.concatenate(gtab_parts))

        main = orders[c][npages - 1]
        rank_main = np.empty(N, dtype=np.int64)
        rank_main[main] = np.arange(NV)
        ctab_parts = []
        for g in range(npages - 1):
            inv_g = np.empty(N, dtype=np.int64)
            inv_g[orders[c][g]] = np.arange(NV)
            perm = np.full((NV_PAD,), NV, dtype=np.int32)
            perm[:NV] = inv_g[main]
            ctab_parts.append(perm)
        ctab16 = (
            _wrap16(np.concatenate(ctab_parts))
            if ctab_parts
            else np.zeros((P, 8), np.int16)
        )

        inv = np.ones((NV_PAD,), dtype=np.float32)
        inv[:NV] = 1.0 / np.maximum(deg[main], 1).astype(np.float32)
        invdeg = inv.reshape(AGG_COLS, P).T.copy()

        nso = np.zeros((IN, NV_PAD), dtype=np.float32)
        nso[:, :NV] = node_scalar[main].T

        per_core.append(dict(gtab=gtab16, ctab=ctab16, invdeg=invdeg, ns_own=nso))

    # shared: transposed node features (bf16) with ones row, padded cols
    ns_t_aug = np.zeros((IN + 1, NS_COLS), dtype=_np_xdt())
    ns_t_aug[:IN, :N] = node_scalar.T.astype(_np_xdt())
    ns_t_aug[IN, :N] = 1.0

    cfg = dict(
        N=N,
        IN=IN,
        NV=NV,
        NODE_CH=NODE_CH,
        NV_PAD=NV_PAD,
        AGG_COLS=AGG_COLS,
        NS_COLS=NS_COLS,
        X_ROWS=X_ROWS,
        PAGE=PAGE,
        npages=npages,
        page_chunks=page_chunks,
        gpieces=gpieces,
        page_tab_cols=page_tab_cols,
        cpieces=cpieces,
        comb_tab_cols=comb_tab_cols,
        n_cores=n_cores,
    )
    shared = dict(ns_t_aug=ns_t_aug)
    return cfg, shared, per_core


def _build(cfg, H=256):
    """Build the Bass/Tile program (shared by all cores)."""
    IN = cfg["IN"]
    NODE_CH = cfg["NODE_CH"]
    NV = cfg["NV"]
    NV_PAD = cfg["NV_PAD"]
    AGG_COLS = cfg["AGG_COLS"]
    NS_COLS = cfg["NS_COLS"]
    X_ROWS = cfg["X_ROWS"]
    PAGE = cfg["PAGE"]
    npages = cfg["npages"]
    page_chunks = cfg["page_chunks"]
    gpieces = cfg["gpieces"]
    page_tab_cols = cfg["page_tab_cols"]
    cpieces = cfg["cpieces"]
    comb_tab_cols = cfg["comb_tab_cols"]
    HH = H // P  # hidden halves (2)
    assert HH == 2

    nc = bacc.Bacc(
        "TRN2", target_bir_lowering=False, debug=False, num_swdge_queues=4
    )

    ns_t = nc.dram_tensor("ns_t", [IN + 1, NS_COLS], _xdt(), kind="ExternalInput")
    ns_own = nc.dram_tensor("ns_own", [IN, NV_PAD], F32, kind="ExternalInput")
    gtab_d = nc.dram_tensor(
        "gtab", [P, sum(page_tab_cols)], I16, kind="ExternalInput"
    )
    ctab_d = nc.dram_tensor(
        "ctab", [P, max(comb_tab_cols * (npages - 1), 8)], I16, kind="ExternalInput"
    )
    invdeg_d = nc.dram_tensor("invdeg", [P, AGG_COLS], F32, kind="ExternalInput")
    w_in_d = nc.dram_tensor("w_in_aug", [IN + 1, H], _xdt(), kind="ExternalInput")
    w_in_f_d = nc.dram_tensor("w_in_f", [IN, H], F32, kind="ExternalInput")
    w_self_d = nc.dram_tensor("w_self", [H, H], F32, kind="ExternalInput")
    w_nei_d = nc.dram_tensor("w_nei", [H, H], F32, kind="ExternalInput")
    b_in_d = nc.dram_tensor("b_in_t", [P, HH], F32, kind="ExternalInput")
    bias_h_d = nc.dram_tensor("bias_h_t", [P, HH], F32, kind="ExternalInput")
    g_out_d = nc.dram_tensor("g_out", [P, HH], F32, kind="ExternalOutput")

    x_pages = [
        nc.dram_tensor(f"x_pg{g}", [PAGE, H], _xdt()) for g in range(npages)
    ]
    spills = [
        nc.dram_tensor(f"agg_pg{g}", [NV_PAD, H], F32) for g in range(npages - 1)
    ]

    with tile.TileContext(nc) as tc:
        with tc.tile_pool(name="const", bufs=1) as cpool:
            w_in_sb = cpool.tile([IN + 1, H], _xdt())
            nc.sync.dma_start(out=w_in_sb[:], in_=w_in_d[:])
            w_in_f_sb = cpool.tile([IN, H], F32)
            nc.sync.dma_start(out=w_in_f_sb[:], in_=w_in_f_d[:])
            w_self_sb = [
                cpool.tile([P, H], F32, tag=f"wself{k}", name=f"wself{k}")
                for k in range(2)
            ]
            w_nei_sb = [
                cpool.tile([P, H], F32, tag=f"wnei{k}", name=f"wnei{k}")
                for k in range(2)
            ]
            for k in range(2):
                nc.sync.dma_start(
                    out=w_self_sb[k][:], in_=w_self_d[k * P : (k + 1) * P, :]
                )
                nc.sync.dma_start(
                    out=w_nei_sb[k][:], in_=w_nei_d[k * P : (k + 1) * P, :]
                )
            b_in_sb = cpool.tile([P, HH], F32)
            bias_h_sb = cpool.tile([P, HH], F32)
            nc.sync.dma_start(out=b_in_sb[:], in_=b_in_d[:])
            nc.sync.dma_start(out=bias_h_sb[:], in_=bias_h_d[:])
            invdeg_sb = cpool.tile([P, AGG_COLS], F32)
            nc.sync.dma_start(out=invdeg_sb[:], in_=invdeg_d[:])
            ident = cpool.tile([P, P], F32)
            from concourse.masks import make_identity

            make_identity(nc, ident[:])

            g_parts = cpool.tile([P, HH, NODE_CH], F32)
            agg = cpool.tile([P, AGG_COLS, H], F32)

            # ---- phases 1+4 fused per page: compute x page, gather,
            #      accumulate, spill; then combine spilled pages ----
            with (
                tc.tile_pool(name="p1", bufs=2) as p1pool,
                tc.tile_pool(name="p1ps", bufs=4, space="PSUM") as p1ps,
                tc.tile_pool(name="g", bufs=5) as gpool,
                tc.tile_pool(name="gc", bufs=2) as gcpool,
                tc.tile_pool(name="gt", bufs=2) as gtpool,
            ):
                tabcol0 = 0
                for g in range(npages):
                    tcols = page_tab_cols[g]
                    if tcols > 0:
                        gtab_sb = gtpool.tile(
                            [P, max(c for c in page_tab_cols)], I16, tag="gtab"
                        )
                        nc.sync.dma_start(
                            out=gtab_sb[:, :tcols],
                            in_=gtab_d[:, tabcol0 : tabcol0 + tcols],
                        )
                    for (col0, lrow) in page_chunks[g]:
                        ns_tile = p1pool.tile([IN + 1, CHUNK1], _xdt(), tag="ns")
                        nc.scalar.dma_start(
                            out=ns_tile[:], in_=ns_t[:, col0 : col0 + CHUNK1]
                        )
                        x_tile = p1pool.tile([P, CHUNK1 // P, H], _xdt(), tag="x")
                        for half in range(CHUNK1 // P // 2):
                            ps = p1ps.tile([P, 2 * H], F32, tag="ps")
                            for t2 in range(2):
                                t = half * 2 + t2
                                nc.tensor.matmul(
                                    out=ps[:, t2 * H : (t2 + 1) * H],
                                    lhsT=ns_tile[:, t * P : (t + 1) * P],
                                    rhs=w_in_sb[:],
                                    start=True,
                                    stop=True,
                                )
                            nc.scalar.activation(
                                out=x_tile[:, half * 2 : half * 2 + 2, :],
                                in_=ps[:],
                                func=RELU,
                            )
                        nc.sync.dma_start(
                            out=x_pages[g][lrow : lrow + CHUNK1, :].rearrange(
                                "(t p) h -> p t h", p=P
                            ),
                            in_=x_tile[:],
                        )
                    if g == 0:
                        nc.vector.memset(agg[:], 0.0)
                    for (pg, tc0, ncols, dcol) in gpieces:
                        if pg != g:
                            continue
                        gbuf = gpool.tile([P, GCOLS, H], _xdt(), tag="gbufh")
                        nidx = ncols * P
                        nc.gpsimd.dma_gather(
                            gbuf[:, 0:ncols, :],
                            x_pages[g][:],
                            gtab_sb[:, tc0 : tc0 + ncols * 8],
                            nidx,
                            nidx,
                            H,
                            single_packet=False,
                        )
                        nc.vector.tensor_tensor(
                            out=agg[:, dcol : dcol + ncols, :],
                            in0=agg[:, dcol : dcol + ncols, :],
                            in1=gbuf[:, 0:ncols, :],
                            op=mybir.AluOpType.add,
                        )
                    tabcol0 += tcols
                    if g < npages - 1:
                        for j0 in range(0, AGG_COLS, GCOLS):
                            jn = min(GCOLS, AGG_COLS - j0)
                            nc.sync.dma_start(
                                out=spills[g][
                                    j0 * P : (j0 + jn) * P, :
                                ].rearrange("(c p) h -> p c h", p=P),
                                in_=agg[:, j0 : j0 + jn, :],
                            )
                            nc.vector.memset(agg[:, j0 : j0 + jn, :], 0.0)

                # combine spilled pages into main-order agg
                if cpieces:
                    ctab_sb = gtpool.tile(
                        [P, comb_tab_cols * (npages - 1)], I16, tag="ctab"
                    )
                    nc.sync.dma_start(
                        out=ctab_sb[:],
                        in_=ctab_d[:, : comb_tab_cols * (npages - 1)],
                    )
                for (g, tc0, ncols, dcol) in cpieces:
                    cbuf = gcpool.tile([P, GCOLS_C, H], F32, tag="gbufc")
                    nidx = ncols * P
                    nc.gpsimd.dma_gather(
                        cbuf[:, 0:ncols, :],
                        spills[g][:, :],
                        ctab_sb[
                            :,
                            g * comb_tab_cols + tc0 : g * comb_tab_cols
                            + tc0
                            + ncols * 8,
                        ],
                        nidx,
                        nidx,
                        H,
                        single_packet=False,
                    )
                    nc.vector.tensor_tensor(
                        out=agg[:, dcol : dcol + ncols, :],
                        in0=agg[:, dcol : dcol + ncols, :],
                        in1=cbuf[:, 0:ncols, :],
                        op=mybir.AluOpType.add,
                    )

            # ---- phase 4b: agg *= 1/deg (per node) ----
            for j in range(AGG_COLS):
                nc.vector.tensor_scalar_mul(
                    out=agg[:, j, :], in0=agg[:, j, :], scalar1=invdeg_sb[:, j : j + 1]
                )

            # ---- phase 5: h^T = relu(W_self^T x^T + W_nei^T agg^T + b); sum ----
            with (
                tc.tile_pool(name="p5", bufs=3) as p5pool,
                tc.tile_pool(name="p5ps", bufs=2, space="PSUM") as p5ps,
                tc.tile_pool(name="p5psh", bufs=3, space="PSUM") as p5psh,
                tc.tile_pool(name="p5pst", bufs=2, space="PSUM") as p5pst,
            ):
                for i in range(NODE_CH):
                    nso = p5pool.tile([IN, CHUNK], F32, tag="nso")
                    nc.scalar.dma_start(
                        out=nso[:], in_=ns_own[:, i * CHUNK : (i + 1) * CHUNK]
                    )
                    xT = [
                        p5pool.tile([P, CHUNK], F32, tag=f"xT{k}", name=f"xT{k}")
                        for k in range(2)
                    ]
                    for kh in range(2):
                        psx = p5ps.tile([P, CHUNK], F32, tag="psx")
                        nc.tensor.matmul(
                            out=psx[:],
                            lhsT=w_in_f_sb[:, kh * P : (kh + 1) * P],
                            rhs=nso[:],
                            start=True,
                            stop=True,
                        )
                        nc.scalar.activation(
                            out=xT[kh][:],
                            in_=psx[:],
                            func=RELU,
                            bias=b_in_sb[:, kh : kh + 1],
                        )
                    aggT = p5pool.tile([P, 2, CHUNK], F32, tag="aggT")
                    for jj in range(CHUNK // P):
                        col = i * (CHUNK // P) + jj
                        pst = p5pst.tile([P, 2, P], F32, tag="pst")
                        for hh in range(2):
                            nc.tensor.transpose(
                                out=pst[:, hh, :],
                                in_=agg[:, col, hh * P : (hh + 1) * P],
                                identity=ident[:],
                            )
                        nc.vector.tensor_copy(
                            out=aggT[:, :, jj * P : (jj + 1) * P], in_=pst[:]
                        )
                    nvalid = min(CHUNK, NV - i * CHUNK)
                    for mh in range(2):
                        psh = p5psh.tile([P, CHUNK], F32, tag="psh")
                        nc.tensor.matmul(
                            out=psh[:],
                            lhsT=w_self_sb[0][:, mh * P : (mh + 1) * P],
                            rhs=xT[0][:],
                            start=True,
                            stop=False,
                        )
                        nc.tensor.matmul(
                            out=psh[:],
                            lhsT=w_self_sb[1][:, mh * P : (mh + 1) * P],
                            rhs=xT[1][:],
                            start=False,
                            stop=False,
                        )
                        nc.tensor.matmul(
                            out=psh[:],
                            lhsT=w_nei_sb[0][:, mh * P : (mh + 1) * P],
                            rhs=aggT[:, 0, :],
                            start=False,
                            stop=False,
                        )
                        nc.tensor.matmul(
                            out=psh[:],
                            lhsT=w_nei_sb[1][:, mh * P : (mh + 1) * P],
                            rhs=aggT[:, 1, :],
                            start=False,
                            stop=True,
                        )
                        hsc = p5pool.tile([P, CHUNK], F32, tag="hsc")
                        nc.scalar.activation(
                            out=hsc[:, :nvalid],
                            in_=psh[:, :nvalid],
                            func=RELU,
                            bias=bias_h_sb[:, mh : mh + 1],
                            accum_out=g_parts[:, mh, i : i + 1],
                        )

            gfin = cpool.tile([P, HH], F32)
            nc.vector.reduce_sum(
                out=gfin[:], in_=g_parts[:], axis=mybir.AxisListType.X
            )
            nc.sync.dma_start(out=g_out_d[:], in_=gfin[:])

    _assign_swdge_queues(nc)
    nc.compile()
    return nc


def _assign_swdge_queues(nc, n_queues=4):
    """Post-schedule: walk the final instruction order, replicate Tile's
    DMASW sem-lane round-robin (mod 8), and pin each Pool DMA's SWDGE queue
    to lane % n_queues so every sem lane sees exactly one queue."""
    lane = 0
    for fn in nc.m.functions:
        for blk in fn.blocks:
            for inst in blk.instructions:
                if inst.engine != mybir.EngineType.Pool:
                    continue
                if isinstance(inst, mybir.InstDMAGatherAnt):
                    inst.queue_num = lane % n_queues
                    lane += 1
                elif isinstance(inst, mybir.InstDMA):
                    lane += 1


def _make_in_maps(cfg, shared, per_core, W_in, b_in, W_self, b_self, W_nei, b_nei):
    H = np.asarray(W_in).shape[1]
    HH = H // P
    w_in_aug = np.concatenate(
        [np.asarray(W_in, np.float32), np.asarray(b_in, np.float32)[None, :]], axis=0
    ).astype(_np_xdt())
    b_in_t = np.asarray(b_in, np.float32).reshape(HH, P).T.copy()
    bias_h = np.asarray(b_self, np.float32) + np.asarray(b_nei, np.float32)
    bias_h_t = bias_h.reshape(HH, P).T.copy()

    in_maps = []
    for c in range(cfg["n_cores"]):
        in_maps.append(
            dict(
                ns_t=shared["ns_t_aug"],
                ns_own=per_core[c]["ns_own"],
                gtab=per_core[c]["gtab"],
                ctab=per_core[c]["ctab"],
                invdeg=per_core[c]["invdeg"],
                w_in_aug=w_in_aug,
                w_in_f=np.asarray(W_in, np.float32),
                w_self=np.asarray(W_self, np.float32),
                w_nei=np.asarray(W_nei, np.float32),
                b_in_t=b_in_t,
                bias_h_t=bias_h_t,
            )
        )
    return in_maps


def _install_ntff_hook_shim():
    """Profiling only: provide antenv.axon_hooks (absent in this image) so
    run_bass_kernel_spmd(trace=True) can reach the NTFF profile hook."""
    import sys
    import types

    try:
        from antenv.axon_hooks import get_axon_ntff_profile_hook  # noqa: F401

        return
    except ImportError:
        pass
    try:
        import antenv
        from trn_agent_boot.trn_boot import _ntff_profile_via_ctypes
    except ImportError:
        return
    mod = types.ModuleType("antenv.axon_hooks")
    holder = {"hook": None}
    mod.set_axon_ntff_profile_hook = lambda h: holder.__setitem__("hook", h)
    mod.get_axon_ntff_profile_hook = lambda: holder["hook"]
    sys.modules["antenv.axon_hooks"] = mod
    antenv.axon_hooks = mod
    try:
        mod.set_axon_ntff_profile_hook(
            _ntff_profile_via_ctypes("/opt/axon/libaxon_pjrt.so")
        )
    except Exception:
        pass


def kernel(
    node_scalar,
    W_in,
    b_in,
    W_self,
    b_self,
    W_nei,
    b_nei,
    W_out,
    b_out,
    edge_index,
):
    from concourse.bass_utils import run_bass_kernel_spmd

    node_scalar = np.asarray(node_scalar, dtype=np.float32)
    N = node_scalar.shape[0]
    H = np.asarray(W_in).shape[1]

    cfg, shared, per_core = _prep(node_scalar, np.asarray(edge_index), N_CORES)
    nc = _build(cfg, H=H)
    in_maps = _make_in_maps(
        cfg, shared, per_core, W_in, b_in, W_self, b_self, W_nei, b_nei
    )

    trace = os.environ.get("KERNEL_TRACE", "0") == "1"
    if trace:
        _install_ntff_hook_shim()
    res = run_bass_kernel_spmd(
        nc, in_maps, core_ids=list(range(N_CORES)), trace=trace
    )
    if trace and res.exec_time_ns is not None:
        print(f"HW exec time: {res.exec_time_ns} ns")

    g_total = np.zeros((H,), dtype=np.float64)
    for c in range(N_CORES):
        out_c = res.results[c]["g_out"]  # (P, HH)
        g_total += out_c.T.reshape(H).astype(np.float64)
    g = (g_total / N).astype(np.float32)
    return (
        g @ np.asarray(W_out, np.float32) + np.asarray(b_out, np.float32)
    ).astype(np.float32)


# revision 22
# speedup vs baseline: 1.0593x; 1.0593x over previous
"""Trainium2 Bass kernel for a GNN message-passing encoder (PocketGraphEncoder).

Math (matches the reference):
    x   = relu(node_scalar @ W_in + b_in)                  # (N, H)
    agg = segment_mean(x[src] over dst, clamp deg>=1)      # (N, H)
    h   = relu(x @ W_self + b_self + agg @ W_nei + b_nei)  # (N, H)
    out = h.mean(0) @ W_out + b_out                        # (OUT,)

Strategy (8 cores, no collectives):
  * Nodes are dealt round-robin from a global in-degree sort, so every core
    owns N/8 nodes with a near-identical degree profile and all cores run one
    shared program (SPMD differs only in input data).
  * Phase 1: every core computes full x (bf16, N rows) into its DRAM, stored
    in "pages" of <=32768 rows so row indices fit int16 for the fast
    dma_gather ucode.  The bias is folded into the matmul via a ones-row.
  * Phase 4 (per page g): core nodes are sorted by their page-g in-degree;
    pass k gathers the k-th page-g in-neighbor's x row for the first M_gk
    nodes.  Degree-sorted order makes destinations a prefix -> scatter-free:
    dma_gather lands rows in accumulator order, DVE adds accumulate.
    Per-page partial sums (in page-g node order) are spilled to DRAM; the
    last page's order IS the main order.  A second tiny dma_gather permutes
    each spilled partial back into main order for the final combine.
  * Phase 5: scale by 1/deg, PE-transpose agg tiles, compute
    h^T = relu(W_self^T x^T + W_nei^T agg^T + b) and reduce along nodes with
    the ACT accumulator -> per-core partial (256,) sum.
  * Host: sum partials, /N, final tiny GEMM with W_out.
"""

import os

import ml_dtypes
import numpy as np

import concourse.bacc as bacc
import concourse.mybir as mybir
import concourse.tile as tile

P = 128
N_CORES = 8
CHUNK = 512  # nodes per phase-5 chunk
CHUNK1 = 1024  # nodes per phase-1 chunk
GCOLS = 16  # gather columns (x128 rows) per dma_gather piece
GCOLS_C = 8  # combine gather piece (f32 staging)
PCAP = 31744  # real x rows per page (multiple of CHUNK1)

F32 = mybir.dt.float32
BF16 = mybir.dt.bfloat16
I16 = mybir.dt.int16
NP_BF16 = ml_dtypes.bfloat16
X_BF16 = True  # store x pages (and phase-1 math) in bf16


def _xdt():
    return BF16 if X_BF16 else F32


def _np_xdt():
    return NP_BF16 if X_BF16 else np.float32

RELU = mybir.ActivationFunctionType.Relu


def _ceil_div(a, b):
    return (a + b - 1) // b


def _wrap16(vals):
    """int16 index list -> [128, n/16] dma_gather table layout."""
    n = len(vals)
    assert n % 16 == 0
    tab = np.asarray(vals, np.int16).reshape(n // 16, 16).T  # [16, n/16]
    return np.tile(tab, (8, 1))  # replicated for the 8 Q7 cores


def _prep(node_scalar, edge_index, n_cores):
    """Host-side preprocessing. Returns (cfg, shared_inputs, per_core_inputs)."""
    N, IN = node_scalar.shape
    src = np.asarray(edge_index[0], dtype=np.int64)
    dst = np.asarray(edge_index[1], dtype=np.int64)

    PAGE = PCAP + CHUNK1  # storage rows per page (<= 32768 for int16)
    assert PAGE <= 32768 and PCAP % CHUNK1 == 0

    deg = np.bincount(dst, minlength=N).astype(np.int64)

    # node -> core: round-robin deal from global total-degree sort
    rank_order = np.argsort(-deg, kind="stable")
    assert N % n_cores == 0
    NV = N // n_cores
    core_nodes = [rank_order[c::n_cores] for c in range(n_cores)]

    NODE_CH = _ceil_div(NV, CHUNK)
    NV_PAD = NODE_CH * CHUNK
    AGG_COLS = NV_PAD // P

    # phase-1 layout: compute x for v in [0, NPAD1); v = N.. are exact zeros
    # (ones-row zeroed there).  npages covers all computed rows.
    N1CH = _ceil_div(N + 1, CHUNK1)
    NPAD1 = N1CH * CHUNK1
    npages = _ceil_div(NPAD1, PCAP)
    X_ROWS = npages * PAGE

    page_of = src // PCAP
    src_local = (src - page_of * PCAP).astype(np.int32)

    # zero row (page-local) per page: prefer a computed-zero row; else a
    # dedicated zero chunk at local PCAP.
    zero_local = []
    zero_chunk_pages = []
    for g in range(npages):
        lo, hi = g * PCAP, (g + 1) * PCAP
        if N < hi and NPAD1 > max(N, lo):
            zero_local.append(max(N, lo) - lo)
        else:
            zero_local.append(PCAP)
            zero_chunk_pages.append(g)

    # phase-1 chunk list: (ns_t column start, x_full storage row start)
    NS_COLS = NPAD1 + len(zero_chunk_pages) * CHUNK1
    page_chunks = [[] for _ in range(npages)]
    for zi, g in enumerate(zero_chunk_pages):
        page_chunks[g].append((NPAD1 + zi * CHUNK1, PCAP))
    for ci in range(N1CH):
        v0 = ci * CHUNK1
        g = v0 // PCAP
        page_chunks[g].append((v0, v0 - g * PCAP))

    # per-(node, page) in-edge lists: group edges by (dst, page)
    key = dst * npages + page_of
    deg_np = np.bincount(key, minlength=N * npages).reshape(N, npages)
    order2 = np.argsort(key, kind="stable")
    src_by_dp = src_local[order2]
    start2 = np.zeros(N * npages + 1, dtype=np.int64)
    np.cumsum(deg_np.reshape(-1), out=start2[1:])

    # per-core, per-page node orders (by page-degree desc; stable)
    # main order = last page's order (its accumulation stays in SBUF).
    orders = []  # [core][page] -> node ids in page order
    for c in range(n_cores):
        cn = core_nodes[c]
        ords = []
        for g in range(npages):
            dg = deg_np[cn, g]
            ords.append(cn[np.argsort(-dg, kind="stable")])
        orders.append(ords)

    # uniform pass lengths per page: M_gk = max over cores of m_gk
    pass_cols = []  # [page][pass] -> cols (128-row units)
    for g in range(npages):
        maxd = 0
        for c in range(n_cores):
            maxd = max(maxd, int(deg_np[core_nodes[c], g].max(initial=0)))
        cols_k = []
        for k in range(maxd):
            m = 0
            for c in range(n_cores):
                m = max(m, int(np.count_nonzero(deg_np[core_nodes[c], g] > k)))
            if m == 0:
                break
            cols_k.append(_ceil_div(m, P))
        pass_cols.append(cols_k)

    # static piece lists (shared by all cores)
    # gather pieces: (page, table col0, ncols(128-row units), agg col0)
    gpieces = []
    page_tab_cols = []  # int16-table column extent per page
    tcol = 0
    for g in range(npages):
        t0 = tcol
        for cols in pass_cols[g]:
            j = 0
            while j < cols:
                nc_ = min(GCOLS, cols - j)
                gpieces.append((g, tcol - t0, nc_, j))
                tcol += nc_ * 8  # 128 idx per col = 8 int16-table cols
                j += nc_
        page_tab_cols.append(tcol - t0)
    # combine pieces: (page, table col0, ncols, agg col0) on the perm table
    cpieces = []
    comb_tab_cols = AGG_COLS * 8
    j = 0
    while j < AGG_COLS:
        nc_ = min(GCOLS_C, AGG_COLS - j)
        for g in range(npages - 1):
            cpieces.append((g, j * 8, nc_, j))
        j += nc_

    # per-core tables + inputs
    per_core = []
    for c in range(n_cores):
        gtab_parts = []
        for g in range(npages):
            zl = zero_local[g]
            cn_g = orders[c][g]
            dg = deg_np[cn_g, g]
            base = start2[cn_g * npages + g]
            for k, cols in enumerate(pass_cols[g]):
                m = int(np.count_nonzero(dg > k))
                block = np.full((cols * P,), zl, dtype=np.int32)
                if m > 0:
                    block[:m] = src_by_dp[base[:m] + k]
                gtab_parts.append(block)
        gtab16 = _wrap16(np.concatenate(gtab_parts))

        main = orders[c][npages - 1]
        rank_main = np.empty(N, dtype=np.int64)
        rank_main[main] = np.arange(NV)
        ctab_parts = []
        for g in range(npages - 1):
            inv_g = np.empty(N, dtype=np.int64)
            inv_g[orders[c][g]] = np.arange(NV)
            perm = np.full((NV_PAD,), NV, dtype=np.int32)
            perm[:NV] = inv_g[main]
            ctab_parts.append(perm)
        ctab16 = (
            _wrap16(np.concatenate(ctab_parts))
            if ctab_parts
            else np.zeros((P, 8), np.int16)
        )

        inv = np.ones((NV_PAD,), dtype=np.float32)
        inv[:NV] = 1.0 / np.maximum(deg[main], 1).astype(np.float32)
        invdeg = inv.reshape(AGG_COLS, P).T.copy()

        nso = np.zeros((IN, NV_PAD), dtype=NP_BF16)
        nso[:, :NV] = node_scalar[main].T.astype(NP_BF16)

        per_core.append(dict(gtab=gtab16, ctab=ctab16, invdeg=invdeg, ns_own=nso))

    # shared: transposed node features (bf16) with ones row, padded cols
    ns_t_aug = np.zeros((IN + 1, NS_COLS), dtype=_np_xdt())
    ns_t_aug[:IN, :N] = node_scalar.T.astype(_np_xdt())
    ns_t_aug[IN, :N] = 1.0

    cfg = dict(
        N=N,
        IN=IN,
        NV=NV,
        NODE_CH=NODE_CH,
        NV_PAD=NV_PAD,
        AGG_COLS=AGG_COLS,
        NS_COLS=NS_COLS,
        X_ROWS=X_ROWS,
        PAGE=PAGE,
        npages=npages,
        page_chunks=page_chunks,
        gpieces=gpieces,
        page_tab_cols=page_tab_cols,
        cpieces=cpieces,
        comb_tab_cols=comb_tab_cols,
        n_cores=n_cores,
    )
    shared = dict(ns_t_aug=ns_t_aug)
    return cfg, shared, per_core


def _build(cfg, H=256):
    """Build the Bass/Tile program (shared by all cores)."""
    IN = cfg["IN"]
    NODE_CH = cfg["NODE_CH"]
    NV = cfg["NV"]
    NV_PAD = cfg["NV_PAD"]
    AGG_COLS = cfg["AGG_COLS"]
    NS_COLS = cfg["NS_COLS"]
    X_ROWS = cfg["X_ROWS"]
    PAGE = cfg["PAGE"]
    npages = cfg["npages"]
    page_chunks = cfg["page_chunks"]
    gpieces = cfg["gpieces"]
    page_tab_cols = cfg["page_tab_cols"]
    cpieces = cfg["cpieces"]
    comb_tab_cols = cfg["comb_tab_cols"]
    HH = H // P  # hidden halves (2)
    assert HH == 2

    nc = bacc.Bacc(
        "TRN2", target_bir_lowering=False, debug=False, num_swdge_queues=4
    )

    ns_t = nc.dram_tensor("ns_t", [IN + 1, NS_COLS], _xdt(), kind="ExternalInput")
    ns_own = nc.dram_tensor("ns_own", [IN, NV_PAD], BF16, kind="ExternalInput")
    gtab_d = nc.dram_tensor(
        "gtab", [P, sum(page_tab_cols)], I16, kind="ExternalInput"
    )
    ctab_d = nc.dram_tensor(
        "ctab", [P, max(comb_tab_cols * (npages - 1), 8)], I16, kind="ExternalInput"
    )
    invdeg_d = nc.dram_tensor("invdeg", [P, AGG_COLS], F32, kind="ExternalInput")
    w_in_d = nc.dram_tensor("w_in_aug", [IN + 1, H], _xdt(), kind="ExternalInput")
    w_self_d = nc.dram_tensor("w_self", [H, H], BF16, kind="ExternalInput")
    w_nei_d = nc.dram_tensor("w_nei", [H, H], BF16, kind="ExternalInput")
    b_in_d = nc.dram_tensor("b_in_t", [P, HH], F32, kind="ExternalInput")
    bias_h_d = nc.dram_tensor("bias_h_t", [P, HH], F32, kind="ExternalInput")
    g_out_d = nc.dram_tensor("g_out", [P, HH], F32, kind="ExternalOutput")

    x_pages = [
        nc.dram_tensor(f"x_pg{g}", [PAGE, H], _xdt()) for g in range(npages)
    ]
    spills = [
        nc.dram_tensor(f"agg_pg{g}", [NV_PAD, H], F32) for g in range(npages - 1)
    ]

    with tile.TileContext(nc) as tc:
        with tc.tile_pool(name="const", bufs=1) as cpool:
            w_in_sb = cpool.tile([IN + 1, H], _xdt())
            nc.sync.dma_start(out=w_in_sb[:], in_=w_in_d[:])
            w_self_sb = [
                cpool.tile([P, H], BF16, tag=f"wself{k}", name=f"wself{k}")
                for k in range(2)
            ]
            w_nei_sb = [
                cpool.tile([P, H], BF16, tag=f"wnei{k}", name=f"wnei{k}")
                for k in range(2)
            ]
            for k in range(2):
                nc.sync.dma_start(
                    out=w_self_sb[k][:], in_=w_self_d[k * P : (k + 1) * P, :]
                )
                nc.sync.dma_start(
                    out=w_nei_sb[k][:], in_=w_nei_d[k * P : (k + 1) * P, :]
                )
            b_in_sb = cpool.tile([P, HH], F32)
            bias_h_sb = cpool.tile([P, HH], F32)
            nc.sync.dma_start(out=b_in_sb[:], in_=b_in_d[:])
            nc.sync.dma_start(out=bias_h_sb[:], in_=bias_h_d[:])
            invdeg_sb = cpool.tile([P, AGG_COLS], F32)
            nc.sync.dma_start(out=invdeg_sb[:], in_=invdeg_d[:])
            ident = cpool.tile([P, P], F32)
            from concourse.masks import make_identity

            make_identity(nc, ident[:])

            g_parts = cpool.tile([P, HH, NODE_CH], F32)
            agg = cpool.tile([P, AGG_COLS, H], F32)

            # ---- phases 1+4 fused per page: compute x page, gather,
            #      accumulate, spill; then combine spilled pages ----
            with (
                tc.tile_pool(name="p1", bufs=2) as p1pool,
                tc.tile_pool(name="p1ps", bufs=4, space="PSUM") as p1ps,
                tc.tile_pool(name="g", bufs=5) as gpool,
                tc.tile_pool(name="gc", bufs=2) as gcpool,
                tc.tile_pool(name="gt", bufs=2) as gtpool,
            ):
                tabcol0 = 0
                for g in range(npages):
                    tcols = page_tab_cols[g]
                    if tcols > 0:
                        gtab_sb = gtpool.tile(
                            [P, max(c for c in page_tab_cols)], I16, tag="gtab"
                        )
                        nc.sync.dma_start(
                            out=gtab_sb[:, :tcols],
                            in_=gtab_d[:, tabcol0 : tabcol0 + tcols],
                        )
                    for (col0, lrow) in page_chunks[g]:
                        ns_tile = p1pool.tile([IN + 1, CHUNK1], _xdt(), tag="ns")
                        nc.scalar.dma_start(
                            out=ns_tile[:], in_=ns_t[:, col0 : col0 + CHUNK1]
                        )
                        x_tile = p1pool.tile([P, CHUNK1 // P, H], _xdt(), tag="x")
                        for half in range(CHUNK1 // P // 2):
                            ps = p1ps.tile([P, 2 * H], F32, tag="ps")
                            for t2 in range(2):
                                t = half * 2 + t2
                                nc.tensor.matmul(
                                    out=ps[:, t2 * H : (t2 + 1) * H],
                                    lhsT=ns_tile[:, t * P : (t + 1) * P],
                                    rhs=w_in_sb[:],
                                    start=True,
                                    stop=True,
                                )
                            nc.scalar.activation(
                                out=x_tile[:, half * 2 : half * 2 + 2, :],
                                in_=ps[:],
                                func=RELU,
                            )
                        nc.sync.dma_start(
                            out=x_pages[g][lrow : lrow + CHUNK1, :].rearrange(
                                "(t p) h -> p t h", p=P
                            ),
                            in_=x_tile[:],
                        )
                    if g == 0:
                        nc.vector.memset(agg[:], 0.0)
                    for (pg, tc0, ncols, dcol) in gpieces:
                        if pg != g:
                            continue
                        gbuf = gpool.tile([P, GCOLS, H], _xdt(), tag="gbufh")
                        nidx = ncols * P
                        nc.gpsimd.dma_gather(
                            gbuf[:, 0:ncols, :],
                            x_pages[g][:],
                            gtab_sb[:, tc0 : tc0 + ncols * 8],
                            nidx,
                            nidx,
                            H,
                            single_packet=False,
                        )
                        nc.vector.tensor_tensor(
                            out=agg[:, dcol : dcol + ncols, :],
                            in0=agg[:, dcol : dcol + ncols, :],
                            in1=gbuf[:, 0:ncols, :],
                            op=mybir.AluOpType.add,
                        )
                    tabcol0 += tcols
                    if g < npages - 1:
                        for j0 in range(0, AGG_COLS, GCOLS):
                            jn = min(GCOLS, AGG_COLS - j0)
                            nc.sync.dma_start(
                                out=spills[g][
                                    j0 * P : (j0 + jn) * P, :
                                ].rearrange("(c p) h -> p c h", p=P),
                                in_=agg[:, j0 : j0 + jn, :],
                            )
                            nc.vector.memset(agg[:, j0 : j0 + jn, :], 0.0)

                # combine spilled pages into main-order agg
                if cpieces:
                    ctab_sb = gtpool.tile(
                        [P, comb_tab_cols * (npages - 1)], I16, tag="ctab"
                    )
                    nc.sync.dma_start(
                        out=ctab_sb[:],
                        in_=ctab_d[:, : comb_tab_cols * (npages - 1)],
                    )
                for (g, tc0, ncols, dcol) in cpieces:
                    cbuf = gcpool.tile([P, GCOLS_C, H], F32, tag="gbufc")
                    nidx = ncols * P
                    nc.gpsimd.dma_gather(
                        cbuf[:, 0:ncols, :],
                        spills[g][:, :],
                        ctab_sb[
                            :,
                            g * comb_tab_cols + tc0 : g * comb_tab_cols
                            + tc0
                            + ncols * 8,
                        ],
                        nidx,
                        nidx,
                        H,
                        single_packet=False,
                    )
                    nc.vector.tensor_tensor(
                        out=agg[:, dcol : dcol + ncols, :],
                        in0=agg[:, dcol : dcol + ncols, :],
                        in1=cbuf[:, 0:ncols, :],
                        op=mybir.AluOpType.add,
                    )

            # ---- phase 4b: agg *= 1/deg (per node) ----
            for j in range(AGG_COLS):
                nc.vector.tensor_scalar_mul(
                    out=agg[:, j, :], in0=agg[:, j, :], scalar1=invdeg_sb[:, j : j + 1]
                )

            # ---- phase 5: h^T = relu(W_self^T x^T + W_nei^T agg^T + b); sum ----
            with (
                tc.tile_pool(name="p5", bufs=3) as p5pool,
                tc.tile_pool(name="p5ps", bufs=2, space="PSUM") as p5ps,
                tc.tile_pool(name="p5psh", bufs=3, space="PSUM") as p5psh,
                tc.tile_pool(name="p5pst", bufs=2, space="PSUM") as p5pst,
            ):
                for i in range(NODE_CH):
                    nso = p5pool.tile([IN, CHUNK], BF16, tag="nso")
                    nc.scalar.dma_start(
                        out=nso[:], in_=ns_own[:, i * CHUNK : (i + 1) * CHUNK]
                    )
                    xT = [
                        p5pool.tile([P, CHUNK], BF16, tag=f"xT{k}", name=f"xT{k}")
                        for k in range(2)
                    ]
                    for kh in range(2):
                        psx = p5ps.tile([P, CHUNK], F32, tag="psx")
                        nc.tensor.matmul(
                            out=psx[:],
                            lhsT=w_in_sb[0:IN, kh * P : (kh + 1) * P],
                            rhs=nso[:],
                            start=True,
                            stop=True,
                        )
                        nc.scalar.activation(
                            out=xT[kh][:],
                            in_=psx[:],
                            func=RELU,
                            bias=b_in_sb[:, kh : kh + 1],
                        )
                    aggT = p5pool.tile([P, 2, CHUNK], BF16, tag="aggT")
                    for jj in range(CHUNK // P):
                        col = i * (CHUNK // P) + jj
                        pst = p5pst.tile([P, 2, P], F32, tag="pst")
                        for hh in range(2):
                            nc.tensor.transpose(
                                out=pst[:, hh, :],
                                in_=agg[:, col, hh * P : (hh + 1) * P],
                                identity=ident[:],
                            )
                        nc.vector.tensor_copy(
                            out=aggT[:, :, jj * P : (jj + 1) * P], in_=pst[:]
                        )
                    nvalid = min(CHUNK, NV - i * CHUNK)
                    for mh in range(2):
                        psh = p5psh.tile([P, CHUNK], F32, tag="psh")
                        nc.tensor.matmul(
                            out=psh[:],
                            lhsT=w_self_sb[0][:, mh * P : (mh + 1) * P],
                            rhs=xT[0][:],
                            start=True,
                            stop=False,
                        )
                        nc.tensor.matmul(
                            out=psh[:],
                            lhsT=w_self_sb[1][:, mh * P : (mh + 1) * P],
                            rhs=xT[1][:],
                            start=False,
                            stop=False,
                        )
                        nc.tensor.matmul(
                            out=psh[:],
                            lhsT=w_nei_sb[0][:, mh * P : (mh + 1) * P],
                            rhs=aggT[:, 0, :],
                            start=False,
                            stop=False,
                        )
                        nc.tensor.matmul(
                            out=psh[:],
                            lhsT=w_nei_sb[1][:, mh * P : (mh + 1) * P],
                            rhs=aggT[:, 1, :],
                            start=False,
                            stop=True,
                        )
                        hsc = p5pool.tile([P, CHUNK], F32, tag="hsc")
                        nc.scalar.activation(
                            out=hsc[:, :nvalid],
                            in_=psh[:, :nvalid],
                            func=RELU,
                            bias=bias_h_sb[:, mh : mh + 1],
                            accum_out=g_parts[:, mh, i : i + 1],
                        )

            gfin = cpool.tile([P, HH], F32)
            nc.vector.reduce_sum(
                out=gfin[:], in_=g_parts[:], axis=mybir.AxisListType.X
            )
            nc.sync.dma_start(out=g_out_d[:], in_=gfin[:])

    _assign_swdge_queues(nc)
    nc.compile()
    return nc


def _assign_swdge_queues(nc, n_queues=4):
    """Post-schedule: walk the final instruction order, replicate Tile's
    DMASW sem-lane round-robin (mod 8), and pin each Pool DMA's SWDGE queue
    to lane % n_queues so every sem lane sees exactly one queue."""
    lane = 0
    for fn in nc.m.functions:
        for blk in fn.blocks:
            for inst in blk.instructions:
                if inst.engine != mybir.EngineType.Pool:
                    continue
                if isinstance(inst, mybir.InstDMAGatherAnt):
                    inst.queue_num = lane % n_queues
                    lane += 1
                elif isinstance(inst, mybir.InstDMA):
                    lane += 1


def _make_in_maps(cfg, shared, per_core, W_in, b_in, W_self, b_self, W_nei, b_nei):
    H = np.asarray(W_in).shape[1]
    HH = H // P
    w_in_aug = np.concatenate(
        [np.asarray(W_in, np.float32), np.asarray(b_in, np.float32)[None, :]], axis=0
    ).astype(_np_xdt())
    b_in_t = np.asarray(b_in, np.float32).reshape(HH, P).T.copy()
    bias_h = np.asarray(b_self, np.float32) + np.asarray(b_nei, np.float32)
    bias_h_t = bias_h.reshape(HH, P).T.copy()

    in_maps = []
    for c in range(cfg["n_cores"]):
        in_maps.append(
            dict(
                ns_t=shared["ns_t_aug"],
                ns_own=per_core[c]["ns_own"],
                gtab=per_core[c]["gtab"],
                ctab=per_core[c]["ctab"],
                invdeg=per_core[c]["invdeg"],
                w_in_aug=w_in_aug,
                w_self=np.asarray(W_self, np.float32).astype(NP_BF16),
                w_nei=np.asarray(W_nei, np.float32).astype(NP_BF16),
                b_in_t=b_in_t,
                bias_h_t=bias_h_t,
            )
        )
    return in_maps


def _install_ntff_hook_shim():
    """Profiling only: provide antenv.axon_hooks (absent in this image) so
    run_bass_kernel_spmd(trace=True) can reach the NTFF profile hook."""
    import sys
    import types

    try:
        from antenv.axon_hooks import get_axon_ntff_profile_hook  # noqa: F401

        return
    except ImportError:
        pass
    try:
        import antenv
        from trn_agent_boot.trn_boot import _ntff_profile_via_ctypes
    except ImportError:
        return
    mod = types.ModuleType("antenv.axon_hooks")
    holder = {"hook": None}
    mod.set_axon_ntff_profile_hook = lambda h: holder.__setitem__("hook", h)
    mod.get_axon_ntff_profile_hook = lambda: holder["hook"]
    sys.modules["antenv.axon_hooks"] = mod
    antenv.axon_hooks = mod
    try:
        mod.set_axon_ntff_profile_hook(
            _ntff_profile_via_ctypes("/opt/axon/libaxon_pjrt.so")
        )
    except Exception:
        pass


def kernel(
    node_scalar,
    W_in,
    b_in,
    W_self,
    b_self,
    W_nei,
    b_nei,
    W_out,
    b_out,
    edge_index,
):
    from concourse.bass_utils import run_bass_kernel_spmd

    node_scalar = np.asarray(node_scalar, dtype=np.float32)
    N = node_scalar.shape[0]
    H = np.asarray(W_in).shape[1]

    cfg, shared, per_core = _prep(node_scalar, np.asarray(edge_index), N_CORES)
    nc = _build(cfg, H=H)
    in_maps = _make_in_maps(
        cfg, shared, per_core, W_in, b_in, W_self, b_self, W_nei, b_nei
    )

    trace = os.environ.get("KERNEL_TRACE", "0") == "1"
    if trace:
        _install_ntff_hook_shim()
    res = run_bass_kernel_spmd(
        nc, in_maps, core_ids=list(range(N_CORES)), trace=trace
    )
    if trace and res.exec_time_ns is not None:
        print(f"HW exec time: {res.exec_time_ns} ns")

    g_total = np.zeros((H,), dtype=np.float64)
    for c in range(N_CORES):
        out_c = res.results[c]["g_out"]  # (P, HH)
        g_total += out_c.T.reshape(H).astype(np.float64)
    g = (g_total / N).astype(np.float32)
    return (
        g @ np.asarray(W_out, np.float32) + np.asarray(b_out, np.float32)
    ).astype(np.float32)
